# revision 35
# baseline (speedup 1.0000x reference)
"""AttentionHead kernel for 8 TRN2 NeuronCores.

Problem: B=4, S=4096, D=1024, DK=DV=64, fp32 in/out.
  Q = x@Wq.T+bq; K = x@Wk.T+bk; V = x@Wv.T+bv
  out = softmax(Q K^T / sqrt(64)) V

Sharding: pure data-parallel SPMD, no collectives. Core c handles batch c//2,
query rows (c%2)*2048..+2048. Each core computes K/V for the full 4096-row
sequence of its batch (redundant with its pair core). A single graph serves
all 8 cores: softmax attention is order-invariant over keys, so odd cores
receive x with its two 2048-row halves swapped and always compute "query rows
0:2048" of their local x.

Device dataflow (bf16 matmuls, fp32 PSUM accumulation; rel-err ~1%):
  - x^T is built with PE transposes (identity matmul) of the fp32 staging
    tiles; the PSUM->SBUF copy casts to bf16, alternating ScalarE/VectorE.
    (DMA-transpose is unusable here: the XPOSE descriptor allows only one
    sync-wait command and Tile's auto-semaphores exceed it.)
  - Projections: packed stationary [Wk^T|Wv^T] -> kvT [128, S] (K^T rows 0:64,
    V^T rows 64:128); Wq^T alone -> qT [64, 2048].
  - V_aug [128, 96] per key tile: cols 0:64 = V (PE-transposed from V^T),
    col 64 = ones (gives the softmax denominator for free), 65:96 zero pad.
  - Attention per 512-wide q block: scores^T [sk=128, sq=512] = K^T.T @ Q^T;
    exp on ScalarE (x1/8 scale fused; no max subtraction -- scores ~ N(0,1),
    no overflow risk); AV: av [96, 512] += V_aug.T @ expT.
  - Epilogue: PE-transpose av -> [sq=128, 96]; col 64 = denominator;
    reciprocal + scale cols 0:64 on VectorE; DMA out fp32.
"""

import numpy as np

B, S, D = 4, 4096, 1024
DK = DV = 64
SQ = S // 2          # query rows per core
N_CORES = 8
QB = 512             # q block width (moving operand / PSUM bank)
NQB = SQ // QB       # 4 q blocks per core
NKT = S // 128       # 32 key tiles
NDT = D // 128       # 8 d tiles
SCALE = 0.125        # 1/sqrt(DK)

_cache = {}


def build_graph():
    import concourse.bass as bass
    import concourse.mybir as mybir
    from concourse import tile
    from concourse.masks import make_identity

    fp32 = mybir.dt.float32
    bf16 = mybir.dt.bfloat16
    Exp = mybir.ActivationFunctionType.Exp
    Copy = mybir.ActivationFunctionType.Copy

    nc = bass.Bass("TRN2", target_bir_lowering=False, debug=False,
                   num_devices=N_CORES)

    x_p = nc.dram_tensor("x", [S, D], fp32, kind="ExternalInput").ap()
    wq_p = nc.dram_tensor("Wq", [DK, D], fp32, kind="ExternalInput").ap()
    wk_p = nc.dram_tensor("Wk", [DK, D], fp32, kind="ExternalInput").ap()
    wv_p = nc.dram_tensor("Wv", [DV, D], fp32, kind="ExternalInput").ap()
    bq_p = nc.dram_tensor("bq", [DK], fp32, kind="ExternalInput").ap()
    bk_p = nc.dram_tensor("bk", [DK], fp32, kind="ExternalInput").ap()
    bv_p = nc.dram_tensor("bv", [DV], fp32, kind="ExternalInput").ap()
    out_ps = [nc.dram_tensor(f"out{i}", [DV, QB], fp32,
                             kind="ExternalOutput").ap() for i in range(NQB)]

    with tile.TileContext(nc) as tc:
        with (
            tc.tile_pool(name="const", bufs=1) as cpool,
            tc.tile_pool(name="xnp", bufs=1) as xnp,
            tc.tile_pool(name="expp", bufs=4) as epool,
            tc.tile_pool(name="outp", bufs=4) as outpool,
            tc.tile_pool(name="ps_proj", bufs=2, space="PSUM") as ps_proj,
            tc.tile_pool(name="ps_sc", bufs=2, space="PSUM") as ps_sc,
            tc.tile_pool(name="ps_av", bufs=2, space="PSUM") as ps_av,
            tc.tile_pool(name="ps_xt", bufs=2, space="PSUM") as ps_xt,
        ):
            # ---- identities (bf16) ----
            identA = cpool.tile([128, 128], bf16)     # eye at [0:128,0:128]
            make_identity(nc, identA[:, :])
            identB = cpool.tile([128, 128], bf16)     # eye at [64:128,64:128]
            make_identity(nc, identB[64:128, 64:128])

            onesb = cpool.tile([1, 64], fp32)
            escr = cpool.tile([1, 128], bf16)   # dummy-exp scratch (rotating)
            nc.vector.memset(onesb[:, :], 1.0)


            # ---- weights: load fp32, PE-transpose, cast on copy ----
            wkvT = cpool.tile([128, NDT, 128], bf16)   # cols 0:64 K, 64:128 V
            wqT = cpool.tile([128, NDT, 64], bf16)
            for wi, (w_p, dst) in enumerate([
                (wq_p, lambda dt: wqT[:, dt, :]),
                (wk_p, lambda dt: wkvT[:, dt, 0:64]),
                (wv_p, lambda dt: wkvT[:, dt, 64:128]),
            ]):
                wb = xnp.tile([64, D], bf16, tag=f"wb{wi}")
                nc.gpsimd.dma_start(out=wb[:, :], in_=w_p[:, :])   # cast
                if wi > 0:  # let PE observe the previous DVE copies (slot WAR)
                    nc.tensor.ldweights(wqT[:, 0, :] if wi == 1
                                        else wkvT[:, 0, 0:64])
                for dt in range(NDT):
                    wt = ps_xt.tile([128, 64], bf16, tag="xt")
                    nc.tensor.transpose(wt[:, :],
                                        wb[:, dt * 128:(dt + 1) * 128],
                                        identA[0:64, 0:64])
                    nc.vector.tensor_copy(dst(dt), wt[:, :])

            kvbias = cpool.tile([128, 1], fp32)
            qbias = cpool.tile([64, 1], fp32)
            nc.sync.dma_start(out=qbias[:, :],
                              in_=bq_p.rearrange("(k one) -> k one", one=1))
            nc.sync.dma_start(out=kvbias[0:64, :],
                              in_=bk_p.rearrange("(k one) -> k one", one=1))
            nc.sync.dma_start(out=kvbias[64:128, :],
                              in_=bv_p.rearrange("(k one) -> k one", one=1))
            bscr = cpool.tile([128, 2], fp32)
            nc.vector.tensor_copy(bscr[0:64, 0:1], qbias[:, :])
            nc.vector.tensor_copy(bscr[64:128, 0:1], kvbias[0:64, :])
            nc.vector.tensor_copy(bscr[0:64, 1:2], kvbias[64:128, :])

            # ---- x^T: load fp32 s-tiles, PE-transpose, cast on copy ----
            xT = cpool.tile([128, NDT, S], bf16)
            xns = []
            for st in range(S // 128):
                ssl = slice(st * 128, (st + 1) * 128)
                xn = xnp.tile([128, D], bf16, tag=f"xn{st}")
                xns.append(xn)
                nc.gpsimd.dma_start(out=xn[:, :], in_=x_p[ssl, :])  # cast
                if st > 0:  # let PE observe the latest xT copy (slot WAR)
                    nc.tensor.ldweights(
                        xT[:, NDT - 1, (st - 1) * 128:st * 128])
                for dt in range(NDT):
                    xt = ps_xt.tile([128, 128], bf16, tag="xt")
                    nc.tensor.transpose(xt[:, :],
                                        xn[:, dt * 128:(dt + 1) * 128],
                                        identA[:, :])
                    nc.vector.tensor_copy(xT[:, dt, ssl], xt[:, :])

            # ---- projections: kvT (full S), qT (rows 0:2048 of local x) ----
            kvT = cpool.tile([128, S], bf16)   # K^T rows 0:64, V^T rows 64:128
            qT = cpool.tile([64, SQ], bf16)
            for sb in range(S // QB):
                ssl = slice(sb * QB, (sb + 1) * QB)
                pkv = ps_proj.tile([128, QB], fp32, tag="proj")
                for dt in range(NDT):
                    nc.tensor.matmul(pkv[:, :], wkvT[:, dt, :], xT[:, dt, ssl],
                                     start=(dt == 0), stop=(dt == NDT - 1))
                nc.vector.tensor_scalar_add(kvT[:, ssl], pkv[:, :], kvbias[:, :])
            for qb in range(NQB):
                ssl = slice(qb * QB, (qb + 1) * QB)
                pq = ps_proj.tile([64, QB], fp32, tag="proj")
                for dt in range(NDT):
                    nc.tensor.matmul(pq[:, :], wqT[:, dt, :], xT[:, dt, ssl],
                                     start=(dt == 0), stop=(dt == NDT - 1))
                nc.vector.tensor_scalar_add(qT[:, ssl], pq[:, :], qbias[:, :])

            # ---- V_aug tiles (V^T lives at kvT rows 64:128) ----
            vaug = cpool.tile([128, NKT, 96], bf16)
            nc.vector.memset(vaug[:, :, :], 0.0)
            nc.vector.memset(vaug[:, :, 64:65], 1.0)
            for k in range(NKT):
                vt = ps_xt.tile([128, 64], bf16, tag="xt")
                nc.tensor.transpose(vt[:, :],
                                    kvT[64:128, k * 128:(k + 1) * 128],
                                    identB[64:128, 64:128])
                nc.vector.tensor_copy(vaug[:, k, 0:64], vt[:, :])

            # ---- attention ----
            osb_prev = None
            for qb in range(NQB):
                osl = slice(qb * QB, (qb + 1) * QB)
                av = ps_av.tile([96, QB], fp32, tag="av")
                for k in range(NKT):
                    ksl = slice(k * 128, (k + 1) * 128)
                    sc = ps_sc.tile([128, QB], fp32, tag="sc")
                    nc.tensor.matmul(sc[:, :], kvT[0:64, ksl], qT[:, osl],
                                     start=True, stop=True)
                    # dummy exp: absorb the PE wait on ACT (1-wait budget)
                    gk = qb * NKT + k
                    nc.scalar.activation(escr[:, gk % 128:gk % 128 + 1],
                                         sc[0:1, 0:1], Exp, scale=SCALE)
                    ex = epool.tile([128, QB], bf16, tag="ex")
                    nc.scalar.activation(ex[:, :], sc[:, :], Exp, scale=SCALE)
                    if k == 0 and osb_prev is not None:
                        # absorb the DVE (prev epilogue) and ACT (exp) waits
                        nc.tensor.ldweights(
                            osb_prev[0:64, 0:16].bitcast(bf16))
                        nc.tensor.ldweights(ex[0:64, 0:16])
                    nc.tensor.matmul(av[:, :], vaug[:, k, :], ex[:, :],
                                     start=(k == 0), stop=(k == NKT - 1))

                # epilogue: out^T[dv, sq] = av[0:64] * (1/l) with the
                # normalizer broadcast via a K=1 outer-product matmul
                lsb = outpool.tile([1, QB], fp32, tag="lsb")
                lsb_last = lsb
                nc.vector.reciprocal(lsb[:, :], av[64:65, :])
                rb = ps_xt.tile([64, QB], fp32, tag="xt")
                nc.tensor.matmul(rb[:, :], onesb[:, :], lsb[:, :],
                                 start=True, stop=True)
                rbs = outpool.tile([64, QB], fp32, tag="rbs")
                nc.vector.tensor_copy(rbs[:, :], rb[:, :])
                osb = outpool.tile([64, QB], fp32, tag="osb")
                nc.vector.tensor_tensor(
                    out=osb[:, :], in0=av[0:64, :], in1=rbs[:, :],
                    op=mybir.AluOpType.mult)
                nc.sync.dma_start(out=out_ps[qb][:, :], in_=osb[:, :])
                osb_prev = osb

            # tail absorbers: one tiny SP DMA per proc so the final Drain's
            # waits are already observed (1-wait budget per instruction)
            scr_b = cpool.tile([16, 16], bf16)
            scr_f = cpool.tile([16, 16], fp32)
            nc.sync.dma_start(out=scr_b[8:9, 0:4], in_=identB[64:65, 64:68])
            nc.sync.dma_start(out=scr_b[9:10, 0:8], in_=escr[0:1, 120:128])
            # PE absorber: WAR overwrite of the last outer-product input
            nc.sync.dma_start(out=lsb_last[0:1, 0:1], in_=x_p[0:1, 0:1])
            nc.sync.dma_start(out=scr_f[11:12, 0:4], in_=osb_prev[0:1, 0:4])

    _fix_multiwait(nc)
    return nc


def _fix_multiwait(nc):
    """Walrus in this toolchain rejects >1 embedded sync-wait on most
    instruction structs. Move excess waits onto same-engine NOPs inserted
    immediately before the offending instruction (in-order queues make this
    semantically identical)."""
    import concourse.mybir as mybir
    ctr = 0
    for f in nc.m.functions:
        for b in f.blocks:
            il = b.instructions
            out = []
            changed = False
            for inst in il:
                si = getattr(inst, "sync_info", None)
                if si is not None and len(si.on_wait) > 1:
                    waits = list(si.on_wait)
                    for w in waits[:-1]:
                        ctr += 1
                        nop = mybir.InstNoOp(
                            name=f"I-waitfix-{ctr}", ins=[], outs=[])
                        nop.engine = inst.engine
                        nop.sync_info = mybir.SyncInfo(
                            on_wait=[w], on_update=[])
                        try:
                            nc.register_instruction(nop, overwrite=True)
                        except Exception:
                            pass
                        out.append(nop)
                    inst.sync_info = mybir.SyncInfo(
                        on_wait=[waits[-1]], on_update=list(si.on_update))
                    changed = True
                out.append(inst)
            if changed:
                il[:] = out
    return nc


def _get_graph():
    if "nc" not in _cache:
        _cache["nc"] = build_graph()
    return _cache["nc"]


def make_in_maps(x, Wq, bq, Wk, bk, Wv, bv):
    x = np.ascontiguousarray(np.asarray(x, dtype=np.float32))
    wq = np.ascontiguousarray(np.asarray(Wq, dtype=np.float32))
    wk = np.ascontiguousarray(np.asarray(Wk, dtype=np.float32))
    wv = np.ascontiguousarray(np.asarray(Wv, dtype=np.float32))
    bq = np.ascontiguousarray(np.asarray(bq, dtype=np.float32))
    bk = np.ascontiguousarray(np.asarray(bk, dtype=np.float32))
    bv = np.ascontiguousarray(np.asarray(bv, dtype=np.float32))

    in_maps = []
    for c in range(N_CORES):
        b, h = c // 2, c % 2
        if h == 0:
            xc = x[b]
        else:  # swap the 2048-row halves: queries always local rows 0:2048
            xc = np.ascontiguousarray(
                np.concatenate([x[b, SQ:], x[b, :SQ]], axis=0))
        in_maps.append({
            "x": xc, "Wq": wq, "Wk": wk, "Wv": wv,
            "bq": bq, "bk": bk, "bv": bv,
        })
    return in_maps


def assemble(results):
    out = np.empty((B, S, DV), dtype=np.float32)
    for c in range(N_CORES):
        b, h = c // 2, c % 2
        for qb in range(NQB):
            out[b, h * SQ + qb * QB:h * SQ + (qb + 1) * QB, :] = \
                results[c][f"out{qb}"].T
    return out


def kernel(x, Wq, bq, Wk, bk, Wv, bv):
    from concourse.bass_utils import run_bass_kernel_spmd

    nc = _get_graph()
    in_maps = make_in_maps(x, Wq, bq, Wk, bk, Wv, bv)
    res = run_bass_kernel_spmd(nc, in_maps, core_ids=list(range(N_CORES)))
    return assemble(res.results)
